# revision 7
# baseline (speedup 1.0000x reference)
"""Multi-head attention (B=2, S=2048, D=1024, H=16, dk=dv=64) on 8 trn2 cores.

Sharding: (batch, head-quad) -> core.  Core i handles batch i//4 and the 4
heads [4*(i%4), 4*(i%4)+4).  Each core computes its partial output
context_h @ W_O[h-slice] summed over its 4 heads; the host sums the 4
partials per batch (the "all-reduce" of the row-sharded output projection).

Per-core kernel (all matmuls bf16 in / fp32 accumulate):
  1. Q^T, K^T projections into [d, s] layout (lhsT = W chunks, rhs = x^T),
     V projection into [s, d] layout (lhsT = x^T chunks, rhs = W_V), with an
     extra ones-column appended per head so the attention A@V matmul also
     produces the softmax denominators for free.
  2. Per (head, 512-query-block): scores^T tiles [sk=128, sq=512] via
     lhsT=K^T tile, rhs=Q^T block (causal: only lower tiles), exp on ACT
     (scale=1/8 folded in, no max-subtraction needed: |logits| <= ~6),
     multiplicative 0/1 causal mask on the diagonal tile groups,
     context^T accumulation via lhsT=V[sk,65], rhs=P^T.
  3. Normalize: reciprocal of the sums row, broadcast across partitions
     (gpsimd partition_broadcast), multiply -> ctx^T bf16.
  4. Output projection: lhsT = ctx^T chunks [128, s-tile], rhs = W_O chunks.
"""

import os
import numpy as np
import ml_dtypes

import concourse.bacc as bacc
import concourse.tile as tile
import concourse.mybir as mybir
import concourse.bass_utils as bass_utils
from concourse.bass import ds

B, S, D, H, DK = 2, 2048, 1024, 16, 64
N_CORES = 8
HPC = 4            # heads per core
NCH = 8            # d-model chunks of 128
NB = 4             # query blocks of 512
BLK = 512
NT = 16            # s tiles of 128
VW = DK + 1        # V columns per head incl. ones column

DT = mybir.dt.bfloat16
NP_DT = ml_dtypes.bfloat16
F32 = mybir.dt.float32

TRACE = False      # set True (or BASS_TRACE=1) to capture an NTFF profile
LAST_RESULTS = None

_CACHED_NC = None


def _build_program():
    nc = bacc.Bacc("TRN2", target_bir_lowering=False, debug=False,
                   enable_asserts=False, num_devices=N_CORES)

    xq_d = nc.dram_tensor("xq_t", [NCH, 128, S], DT, kind="ExternalInput")
    xk_d = nc.dram_tensor("xk_t", [NCH, 128, S], DT, kind="ExternalInput")
    xv_d = nc.dram_tensor("xv_t", [NCH, 128, S], DT, kind="ExternalInput")
    wq_d = nc.dram_tensor("wq", [NCH, 128, HPC * DK], DT, kind="ExternalInput")
    wk_d = nc.dram_tensor("wk", [NCH, 128, HPC * DK], DT, kind="ExternalInput")
    wv_d = nc.dram_tensor("wv", [NCH, 128, HPC * DK], DT, kind="ExternalInput")
    wo_d = nc.dram_tensor("wo", [2, 128, D], DT, kind="ExternalInput")
    mask_d = nc.dram_tensor("mask01", [128, S], DT, kind="ExternalInput")
    out_d = nc.dram_tensor("out_partial", [S, D], F32, kind="ExternalOutput")

    with tile.TileContext(nc) as tc:
        _body(tc, xq_d, xk_d, xv_d, wq_d, wk_d, wv_d, wo_d, mask_d, out_d)
    nc.compile()
    return nc


def _body(tc, xq_d, xk_d, xv_d, wq_d, wk_d, wv_d, wo_d, mask_d, out_d):
    nc = tc.nc
    EXP = mybir.ActivationFunctionType.Exp
    MUL = mybir.AluOpType.mult

    with (
        tc.tile_pool(name="consts", bufs=1) as consts,
        tc.tile_pool(name="persist", bufs=1) as persist,
        tc.tile_pool(name="pt", bufs=4) as pt_pool,
        tc.tile_pool(name="osb", bufs=3) as out_pool,
        tc.tile_pool(name="small", bufs=3) as small,
    ):
        # ---- constants ----
        wq_sb = consts.tile([128, NCH, HPC * DK], DT)
        wk_sb = consts.tile([128, NCH, HPC * DK], DT)
        wv_sb = consts.tile([128, NCH, HPC * DK], DT)
        wo_sb = consts.tile([128, 2, D], DT)
        mask_sb = consts.tile([128, S], DT)
        nc.sync.dma_start(wq_sb[:], wq_d.rearrange("c p n -> p c n"))
        nc.sync.dma_start(wk_sb[:], wk_d.rearrange("c p n -> p c n"))
        nc.sync.dma_start(wv_sb[:], wv_d.rearrange("c p n -> p c n"))

        # ---- persistent activations ----
        qt_sb = persist.tile([128, 2, S], DT)        # Q^T, pair-major
        kt_sb = persist.tile([128, 2, S], DT)        # K^T
        v_sb = persist.tile([128, NT, HPC * VW], DT)  # V + ones cols
        ctxt_sb = persist.tile([128, 2, S], DT)      # context^T

        for hh in range(HPC):
            nc.vector.memset(v_sb[:, :, hh * VW + DK: hh * VW + DK + 1], 1.0)

        # ---- projections ----
        with (
            tc.tile_pool(name="xt", bufs=1) as xt_pool,
            tc.tile_pool(name="psum_proj", bufs=4, space="PSUM") as proj_pool,
        ):
            xq_sb = xt_pool.tile([128, NCH, S], DT)
            xk_sb = xt_pool.tile([128, NCH, S], DT)
            xv_sb = xt_pool.tile([128, NCH, S], DT)
            for c in range(NCH):
                nc.sync.dma_start(xq_sb[:, c, :], xq_d[c])
            for c in range(NCH):
                nc.sync.dma_start(xk_sb[:, c, :], xk_d[c])
            for c in range(NCH):
                nc.sync.dma_start(xv_sb[:, c, :], xv_d[c])
            nc.sync.dma_start(wo_sb[:], wo_d.rearrange("c p n -> p c n"))
            nc.sync.dma_start(mask_sb[:], mask_d[:])

            # Q^T / K^T: out[m=pair cols 128, n=sq 512]
            for dst, w_sb, x_sb in ((qt_sb, wq_sb, xq_sb), (kt_sb, wk_sb, xk_sb)):
                for p in range(2):
                    for blk in range(NB):
                        ps = proj_pool.tile([128, BLK], F32, tag="qk")
                        for c in range(NCH):
                            nc.tensor.matmul(
                                ps[:],
                                lhsT=w_sb[:, c, ds(128 * p, 128)],
                                rhs=x_sb[:, c, ds(BLK * blk, BLK)],
                                start=(c == 0), stop=(c == NCH - 1))
                        nc.vector.tensor_copy(dst[:, p, ds(BLK * blk, BLK)], ps[:])

            # V: out[m=s-tile 128, n=4 heads x 64]
            for t in range(NT):
                ps = proj_pool.tile([128, HPC * DK], F32, tag="v")
                for c in range(NCH):
                    nc.tensor.matmul(
                        ps[:],
                        lhsT=xv_sb[:, c, ds(128 * t, 128)],
                        rhs=wv_sb[:, c, :],
                        start=(c == 0), stop=(c == NCH - 1))
                dst = v_sb[:, t, :].rearrange("p (hh e) -> p hh e", hh=HPC)[:, :, 0:DK]
                src = ps[:].rearrange("p (hh e) -> p hh e", hh=HPC)
                nc.scalar.copy(dst, src)

        # ---- attention + output projection, interleaved per query block ----
        with (
            tc.tile_pool(name="psum_sc", bufs=2, space="PSUM") as sc_pool,
            tc.tile_pool(name="psum_ctx", bufs=2, space="PSUM") as ctx_pool,
            tc.tile_pool(name="psum_out", bufs=2, space="PSUM") as po_pool,
        ):
            _attention(tc, qt_sb, kt_sb, v_sb, ctxt_sb, mask_sb, wo_sb,
                       sc_pool, ctx_pool, po_pool, pt_pool, out_pool, small,
                       out_d)


def _attention(tc, qt_sb, kt_sb, v_sb, ctxt_sb, mask_sb, wo_sb,
               sc_pool, ctx_pool, po_pool, pt_pool, out_pool, small, out_d):
    nc = tc.nc
    EXP = mybir.ActivationFunctionType.Exp
    MUL = mybir.AluOpType.mult
    if True:
        for blk in range(NB):
            for h in range(HPC):
                hp, h2 = h // 2, h % 2
                ctxp = ctx_pool.tile([128, BLK], F32)
                ngr = 2 * (blk + 1)  # groups of 2 sk-tiles each
                for g in range(ngr):
                    sc = sc_pool.tile([128, 1024], F32)
                    for j in range(2):
                        skt = 2 * g + j
                        nc.tensor.matmul(
                            sc[:, ds(512 * j, 512)],
                            lhsT=kt_sb[ds(64 * h2, 64), hp, ds(128 * skt, 128)],
                            rhs=qt_sb[ds(64 * h2, 64), hp, ds(BLK * blk, BLK)],
                            start=True, stop=True)
                    pt = pt_pool.tile([128, 1024], DT)
                    nc.scalar.activation(pt[:], sc[:], EXP, scale=0.125)
                    if g >= 2 * blk:  # diagonal groups: zero the masked region
                        u = g - 2 * blk
                        nc.vector.tensor_tensor(
                            pt[:], pt[:], mask_sb[:, ds(1024 * u, 1024)], MUL)
                    for j in range(2):
                        skt = 2 * g + j
                        nc.tensor.matmul(
                            ctxp[0:DK + 1, :],
                            lhsT=v_sb[:, skt, ds(h * VW, VW)],
                            rhs=pt[:, ds(512 * j, 512)],
                            start=(g == 0 and j == 0),
                            stop=(g == ngr - 1 and j == 1))
                # normalize: rows 0..63 scaled by 1/row64, write ctx^T bf16.
                # (copy the sums row out on ACT — cheap there; broadcast it
                # across partitions on gpsimd; reciprocal on 64 DVE lanes —
                # a [1,512] DVE op would serialize on a single lane.)
                sums = small.tile([1, BLK], F32, tag="sums")
                nc.scalar.copy(sums[:], ctxp[ds(DK, 1), :])
                bcr = small.tile([64, BLK], F32, tag="bcr")
                nc.gpsimd.partition_broadcast(bcr[:], sums[:])
                bc = small.tile([64, BLK], F32, tag="bc")
                nc.vector.reciprocal(bc[:], bcr[:])
                nc.vector.tensor_tensor(
                    ctxt_sb[ds(64 * h2, 64), hp, ds(BLK * blk, BLK)],
                    ctxp[0:64, :], bc[:], MUL)

            for t in range(4 * blk, 4 * blk + 4):
                ob = out_pool.tile([128, D], F32)
                for nb in range(2):
                    pp = po_pool.tile([128, 512], F32)
                    for cc in range(2):
                        nc.tensor.matmul(
                            pp[:],
                            lhsT=ctxt_sb[:, cc, ds(128 * t, 128)],
                            rhs=wo_sb[:, cc, ds(512 * nb, 512)],
                            start=(cc == 0), stop=(cc == 1))
                    nc.vector.tensor_copy(ob[:, ds(512 * nb, 512)], pp[:])
                nc.sync.dma_start(out_d[ds(128 * t, 128), :], ob[:])


def _make_mask():
    # mask01[i, 512*m + q] = 1.0 iff key (128*m + i) <= query q  (within the
    # diagonal 512-block; m = sk-tile offset within the block)
    i = np.arange(128)[:, None]
    q = np.arange(512)[None, :]
    cols = [(128 * m + i <= q) for m in range(4)]
    return np.concatenate(cols, axis=1).astype(NP_DT)


def _prep_core_inputs(inputs, core):
    b = core // 4
    h0 = HPC * (core % 4)
    c0, c1 = h0 * DK, (h0 + HPC) * DK
    f32 = np.float32

    def t_chunks(x):  # [S, D] -> [NCH, 128, S]
        xt = np.ascontiguousarray(np.asarray(x, f32).T)
        return xt.reshape(NCH, 128, S).astype(NP_DT)

    return {
        "xq_t": t_chunks(inputs["input_Q"][b]),
        "xk_t": t_chunks(inputs["input_K"][b]),
        "xv_t": t_chunks(inputs["input_V"][b]),
        "wq": np.asarray(inputs["W_Q"], f32)[:, c0:c1].reshape(NCH, 128, HPC * DK).astype(NP_DT),
        "wk": np.asarray(inputs["W_K"], f32)[:, c0:c1].reshape(NCH, 128, HPC * DK).astype(NP_DT),
        "wv": np.asarray(inputs["W_V"], f32)[:, c0:c1].reshape(NCH, 128, HPC * DK).astype(NP_DT),
        "wo": np.ascontiguousarray(np.asarray(inputs["W_O"], f32)[c0:c1, :]).reshape(2, 128, D).astype(NP_DT),
        "mask01": _make_mask(),
    }


def get_program():
    global _CACHED_NC
    if _CACHED_NC is None:
        _CACHED_NC = _build_program()
    return _CACHED_NC


def kernel(**inputs):
    global LAST_RESULTS
    nc = get_program()
    in_maps = [_prep_core_inputs(inputs, core) for core in range(N_CORES)]
    res = bass_utils.run_bass_kernel_spmd(
        nc, in_maps, core_ids=list(range(N_CORES)),
        trace=TRACE or bool(int(os.environ.get("BASS_TRACE", "0") or 0)))
    LAST_RESULTS = res
    out = np.zeros((B, S, D), np.float32)
    for core in range(N_CORES):
        out[core // 4] += res.results[core]["out_partial"]
    return out


# revision 14
# speedup vs baseline: 1.0988x; 1.0988x over previous
"""Multi-head attention (B=2, S=2048, D=1024, H=16, dk=dv=64) on 8 trn2 cores.

Sharding: (batch, head-quad) -> core.  Core i handles batch i//4 and the 4
heads [4*(i%4), 4*(i%4)+4).  Each core computes its partial output
context_h @ W_O[h-slice] summed over its 4 heads; the host sums the 4
partials per batch (the "all-reduce" of the row-sharded output projection).

Per-core kernel (all matmuls bf16 in / fp32 accumulate):
  1. Q^T, K^T projections into [d, s] layout (lhsT = W chunks, rhs = x^T),
     V projection into [s, d] layout (lhsT = x^T chunks, rhs = W_V), with an
     extra ones-column appended per head so the attention A@V matmul also
     produces the softmax denominators for free.
  2. Per (head, 512-query-block): scores^T tiles [sk=128, sq=512] via
     lhsT=K^T tile, rhs=Q^T block (causal: only lower tiles), exp on ACT
     (scale=1/8 folded in, no max-subtraction needed: |logits| <= ~6),
     multiplicative 0/1 causal mask on the diagonal tile groups,
     context^T accumulation via lhsT=V[sk,65], rhs=P^T.
  3. Normalize: reciprocal of the sums row, broadcast across partitions
     (gpsimd partition_broadcast), multiply -> ctx^T bf16.
  4. Output projection: lhsT = ctx^T chunks [128, s-tile], rhs = W_O chunks.
"""

import os
import numpy as np
import ml_dtypes

import concourse.bacc as bacc
import concourse.tile as tile
import concourse.mybir as mybir
import concourse.bass_utils as bass_utils
from concourse.bass import ds

B, S, D, H, DK = 2, 2048, 1024, 16, 64
N_CORES = 8
HPC = 4            # heads per core
NCH = 8            # d-model chunks of 128
NB = 4             # query blocks of 512
BLK = 512
NT = 16            # s tiles of 128
VW = DK + 1        # V columns per head incl. ones column

DT = mybir.dt.bfloat16
NP_DT = ml_dtypes.bfloat16
F32 = mybir.dt.float32

TRACE = False      # set True (or BASS_TRACE=1) to capture an NTFF profile
LAST_RESULTS = None

_CACHED_NC = None


def _build_program():
    nc = bacc.Bacc("TRN2", target_bir_lowering=False, debug=False,
                   enable_asserts=False, num_devices=N_CORES)

    xq_d = nc.dram_tensor("xq_t", [NCH, 128, S], DT, kind="ExternalInput")
    xk_d = nc.dram_tensor("xk_t", [NCH, 128, S], DT, kind="ExternalInput")
    xv_d = nc.dram_tensor("xv_t", [NCH, 128, S], DT, kind="ExternalInput")
    wq_d = nc.dram_tensor("wq", [NCH, 128, HPC * DK], DT, kind="ExternalInput")
    wk_d = nc.dram_tensor("wk", [NCH, 128, HPC * DK], DT, kind="ExternalInput")
    wv_d = nc.dram_tensor("wv", [NCH, 128, HPC * DK], DT, kind="ExternalInput")
    wo_d = nc.dram_tensor("wo", [2, 128, D], DT, kind="ExternalInput")
    mask_d = nc.dram_tensor("mask01", [128, S], DT, kind="ExternalInput")
    out_d = nc.dram_tensor("out_partial", [S, D], F32, kind="ExternalOutput")

    with tile.TileContext(nc) as tc:
        _body(tc, xq_d, xk_d, xv_d, wq_d, wk_d, wv_d, wo_d, mask_d, out_d)
    nc.compile()
    return nc


def _body(tc, xq_d, xk_d, xv_d, wq_d, wk_d, wv_d, wo_d, mask_d, out_d):
    nc = tc.nc
    EXP = mybir.ActivationFunctionType.Exp
    MUL = mybir.AluOpType.mult

    with (
        tc.tile_pool(name="consts", bufs=1) as consts,
        tc.tile_pool(name="persist", bufs=1) as persist,
        tc.tile_pool(name="pt", bufs=6) as pt_pool,
        tc.tile_pool(name="osb", bufs=3) as out_pool,
        tc.tile_pool(name="small", bufs=3) as small,
    ):
        # ---- constants ----
        wq_sb = consts.tile([128, NCH, HPC * DK], DT)
        wk_sb = consts.tile([128, NCH, HPC * DK], DT)
        wv_sb = consts.tile([128, NCH, HPC * DK], DT)
        wo_sb = consts.tile([128, 2, D], DT)
        mask_sb = consts.tile([128, S], DT)
        nc.sync.dma_start(wq_sb[:], wq_d.rearrange("c p n -> p c n"))
        nc.sync.dma_start(wk_sb[:], wk_d.rearrange("c p n -> p c n"))
        nc.sync.dma_start(wv_sb[:], wv_d.rearrange("c p n -> p c n"))

        # ---- persistent activations ----
        qt_sb = persist.tile([128, 2, S], DT)        # Q^T, pair-major
        kt_sb = persist.tile([128, 2, S], DT)        # K^T
        v_sb = persist.tile([128, NT, HPC * VW], DT)  # V + ones cols
        ctxt_sb = persist.tile([128, 2, S], DT)      # context^T

        for hh in range(HPC):
            nc.vector.memset(v_sb[:, :, hh * VW + DK: hh * VW + DK + 1], 1.0)

        # ---- projections ----
        with (
            tc.tile_pool(name="xt", bufs=1) as xt_pool,
            tc.tile_pool(name="psum_proj", bufs=4, space="PSUM") as proj_pool,
        ):
            xq_sb = xt_pool.tile([128, NCH, S], DT)
            xk_sb = xt_pool.tile([128, NCH, S], DT)
            xv_sb = xt_pool.tile([128, NCH, S], DT)
            for c in range(NCH):
                nc.sync.dma_start(xq_sb[:, c, :], xq_d[c])
            for c in range(NCH):
                nc.sync.dma_start(xk_sb[:, c, :], xk_d[c])
            for c in range(NCH):
                nc.sync.dma_start(xv_sb[:, c, :], xv_d[c])
            nc.sync.dma_start(wo_sb[:], wo_d.rearrange("c p n -> p c n"))
            nc.sync.dma_start(mask_sb[:], mask_d[:])

            # Q^T / K^T: out[m=pair cols 128, n=sq 512]
            for dst, w_sb, x_sb in ((qt_sb, wq_sb, xq_sb), (kt_sb, wk_sb, xk_sb)):
                for p in range(2):
                    for blk in range(NB):
                        ps = proj_pool.tile([128, BLK], F32, tag="qk")
                        for c in range(NCH):
                            nc.tensor.matmul(
                                ps[:],
                                lhsT=w_sb[:, c, ds(128 * p, 128)],
                                rhs=x_sb[:, c, ds(BLK * blk, BLK)],
                                start=(c == 0), stop=(c == NCH - 1))
                        nc.vector.tensor_copy(dst[:, p, ds(BLK * blk, BLK)], ps[:])

            # V: out[m=s-tile 128, n=4 heads x 64]
            for t in range(NT):
                ps = proj_pool.tile([128, HPC * DK], F32, tag="v")
                for c in range(NCH):
                    nc.tensor.matmul(
                        ps[:],
                        lhsT=xv_sb[:, c, ds(128 * t, 128)],
                        rhs=wv_sb[:, c, :],
                        start=(c == 0), stop=(c == NCH - 1))
                dst = v_sb[:, t, :].rearrange("p (hh e) -> p hh e", hh=HPC)[:, :, 0:DK]
                src = ps[:].rearrange("p (hh e) -> p hh e", hh=HPC)
                nc.vector.tensor_copy(dst, src)

        # ---- attention + output projection, interleaved per query block ----
        with (
            tc.tile_pool(name="psum_sc", bufs=2, space="PSUM") as sc_pool,
            tc.tile_pool(name="psum_ctx", bufs=2, space="PSUM") as ctx_pool,
            tc.tile_pool(name="psum_out", bufs=2, space="PSUM") as po_pool,
        ):
            _attention(tc, qt_sb, kt_sb, v_sb, ctxt_sb, mask_sb, wo_sb,
                       sc_pool, ctx_pool, po_pool, pt_pool, out_pool, small,
                       out_d)


def _attention(tc, qt_sb, kt_sb, v_sb, ctxt_sb, mask_sb, wo_sb,
               sc_pool, ctx_pool, po_pool, pt_pool, out_pool, small, out_d):
    nc = tc.nc
    EXP = mybir.ActivationFunctionType.Exp
    MUL = mybir.AluOpType.mult
    if True:
        for blk in range(NB):
            ngr = 2 * (blk + 1)  # groups of 2 sk-tiles each
            for hp in range(2):
                # Interleave the two heads of the pair and run the A@V
                # consumer 3 tasks behind the scores/exp producer so the
                # in-order PE queue never waits on ACT's exp latency.
                ctxps = {}
                queue = []  # (h, g, pt)

                def consume():
                    h, g, pt = queue.pop(0)
                    for j in range(2):
                        skt = 2 * g + j
                        nc.tensor.matmul(
                            ctxps[h][0:DK + 1, :],
                            lhsT=v_sb[:, skt, ds(h * VW, VW)],
                            rhs=pt[:, ds(512 * j, 512)],
                            start=(g == 0 and j == 0),
                            stop=(g == ngr - 1 and j == 1))

                for g in range(ngr):
                    for h2 in range(2):
                        h = 2 * hp + h2
                        if g == 0:
                            ctxps[h] = ctx_pool.tile(
                                [128, BLK], F32, name=f"ctx{h2}", tag="ctx")
                        sc = sc_pool.tile([128, 1024], F32)
                        for j in range(2):
                            skt = 2 * g + j
                            nc.tensor.matmul(
                                sc[:, ds(512 * j, 512)],
                                lhsT=kt_sb[ds(64 * h2, 64), hp, ds(128 * skt, 128)],
                                rhs=qt_sb[ds(64 * h2, 64), hp, ds(BLK * blk, BLK)],
                                start=True, stop=True)
                        pt = pt_pool.tile([128, 1024], DT)
                        nc.scalar.activation(pt[:], sc[:], EXP, scale=0.125)
                        if g >= 2 * blk:  # diagonal groups: zero masked region
                            u = g - 2 * blk
                            nc.vector.tensor_tensor(
                                pt[:], pt[:], mask_sb[:, ds(1024 * u, 1024)], MUL)
                        queue.append((h, g, pt))
                        while len(queue) > 3:
                            consume()
                while queue:
                    consume()
                # normalize: rows 0..63 scaled by 1/row64, write ctx^T bf16
                for h2 in range(2):
                    h = 2 * hp + h2
                    # custom-DVE ops read garbage from PSUM -> copy sums row
                    # to SBUF on ACT first (cheap: ACT cost ~ free size only)
                    sums = small.tile([1, BLK], F32, tag="sums")
                    nc.scalar.copy(sums[:], ctxps[h][ds(DK, 1), :])
                    r = small.tile([1, BLK], F32, tag="r")
                    nc.vector.reciprocal_approx_fast(out=r[:], in_=sums[:])
                    bc = small.tile([64, BLK], F32, tag="bc")
                    nc.gpsimd.partition_broadcast(bc[:], r[:])
                    nc.vector.tensor_tensor(
                        ctxt_sb[ds(64 * h2, 64), hp, ds(BLK * blk, BLK)],
                        ctxps[h][0:64, :], bc[:], MUL)

            for t in range(4 * blk, 4 * blk + 4):
                ob = out_pool.tile([128, D], F32)
                for nb in range(2):
                    pp = po_pool.tile([128, 512], F32)
                    for cc in range(2):
                        nc.tensor.matmul(
                            pp[:],
                            lhsT=ctxt_sb[:, cc, ds(128 * t, 128)],
                            rhs=wo_sb[:, cc, ds(512 * nb, 512)],
                            start=(cc == 0), stop=(cc == 1))
                    nc.vector.tensor_copy(ob[:, ds(512 * nb, 512)], pp[:])
                nc.sync.dma_start(out_d[ds(128 * t, 128), :], ob[:])


def _make_mask():
    # mask01[i, 512*m + q] = 1.0 iff key (128*m + i) <= query q  (within the
    # diagonal 512-block; m = sk-tile offset within the block)
    i = np.arange(128)[:, None]
    q = np.arange(512)[None, :]
    cols = [(128 * m + i <= q) for m in range(4)]
    return np.concatenate(cols, axis=1).astype(NP_DT)


def _prep_core_inputs(inputs, core):
    b = core // 4
    h0 = HPC * (core % 4)
    c0, c1 = h0 * DK, (h0 + HPC) * DK
    f32 = np.float32

    def t_chunks(x):  # [S, D] -> [NCH, 128, S]
        xt = np.ascontiguousarray(np.asarray(x, f32).T)
        return xt.reshape(NCH, 128, S).astype(NP_DT)

    return {
        "xq_t": t_chunks(inputs["input_Q"][b]),
        "xk_t": t_chunks(inputs["input_K"][b]),
        "xv_t": t_chunks(inputs["input_V"][b]),
        "wq": np.asarray(inputs["W_Q"], f32)[:, c0:c1].reshape(NCH, 128, HPC * DK).astype(NP_DT),
        "wk": np.asarray(inputs["W_K"], f32)[:, c0:c1].reshape(NCH, 128, HPC * DK).astype(NP_DT),
        "wv": np.asarray(inputs["W_V"], f32)[:, c0:c1].reshape(NCH, 128, HPC * DK).astype(NP_DT),
        "wo": np.ascontiguousarray(np.asarray(inputs["W_O"], f32)[c0:c1, :]).reshape(2, 128, D).astype(NP_DT),
        "mask01": _make_mask(),
    }


def get_program():
    global _CACHED_NC
    if _CACHED_NC is None:
        _CACHED_NC = _build_program()
    return _CACHED_NC


def kernel(**inputs):
    global LAST_RESULTS
    nc = get_program()
    in_maps = [_prep_core_inputs(inputs, core) for core in range(N_CORES)]
    res = bass_utils.run_bass_kernel_spmd(
        nc, in_maps, core_ids=list(range(N_CORES)),
        trace=TRACE or bool(int(os.environ.get("BASS_TRACE", "0") or 0)))
    LAST_RESULTS = res
    out = np.zeros((B, S, D), np.float32)
    for core in range(N_CORES):
        out[core // 4] += res.results[core]["out_partial"]
    return out
